# revision 6
# baseline (speedup 1.0000x reference)
"""MoE expert-FFN (grouped GEMM + SwiGLU) kernel for 8 Trainium2 NeuronCores.

Expert-parallel: expert n lives on core n (N=8 experts, 8 cores).
Host does dispatch (gather tokens per expert) and combine (scatter back);
each core runs lin1 -> SwiGLU -> lin2 -> router-scale for its expert.

Per-core device program (all fp32):
  phase 1: abT[j] = Wl1_cols_j^T @ xg^T accumulated in PSUM per
           (col-block j, token-block m); SwiGLU fuses the a/b halves:
           hT = silu(bT) * aT, spilled to a DRAM scratch [H, Mi].
  phase 2: y[m, :] = (hT[:, m]^T @ Wl2) * sg[m], written [Mi, D].
"""

import sys

sys.path.insert(0, "/opt/trn_rl_repo")

import numpy as np

M, D, H, N_EXP, TOPK = 8192, 2048, 2048, 8, 2
MI = M * TOPK // N_EXP  # 2048 tokens per expert (balanced routing)
P = 128
KT = D // P    # 16 contraction tiles in lin1 (over D)
JT = H // P    # 16 column blocks per a/b half of Wl1
MB = MI // 512  # 4 moving-operand blocks of 512 tokens (phase 1)
DB = D // 512   # 4 output d-blocks of 512 (phase 2)
MT = MI // P   # 16 token tiles (phase 2)

_CACHE = {}


def _build_nc():
    from concourse import bacc, mybir
    import concourse.tile as tile

    f32 = mybir.dt.float32
    f32r = mybir.dt.float32r
    SIGMOID = mybir.ActivationFunctionType.Sigmoid

    nc = bacc.Bacc(None, target_bir_lowering=False)
    xgT = nc.dram_tensor("xgT", [D, MI], f32r, kind="ExternalInput")
    # w1t[j, kk, k*128+nn] = Wl1[k*128+kk, j*128+nn]; j<16 -> gate (a), j>=16 -> up (b)
    w1t = nc.dram_tensor("w1t", [2 * JT, P, KT * P], f32r, kind="ExternalInput")
    w2 = nc.dram_tensor("w2", [H, D], f32r, kind="ExternalInput")
    sg = nc.dram_tensor("sg", [P, MT], f32, kind="ExternalInput")
    y = nc.dram_tensor("y", [MI, D], f32, kind="ExternalOutput")
    hT = nc.dram_tensor("hT", [H, MI], f32r)  # internal scratch

    with tile.TileContext(nc) as tc:
        with tc.tile_pool(name="xg", bufs=KT) as xgp, \
             tc.tile_pool(name="w1", bufs=3) as w1p, \
             tc.tile_pool(name="ht", bufs=2) as htp, \
             tc.tile_pool(name="sil", bufs=4) as silp, \
             tc.tile_pool(name="ps1", bufs=8, space="PSUM") as pp1:
            xg_tiles = []
            for k in range(KT):
                t = xgp.tile([P, MI], f32r, tag="xg")
                nc.sync.dma_start(out=t, in_=xgT[k * P:(k + 1) * P, :])
                xg_tiles.append(t)
            for j in range(JT):
                w1a = w1p.tile([P, KT * P], f32r, tag="w1")
                nc.sync.dma_start(out=w1a, in_=w1t[j])
                w1b = w1p.tile([P, KT * P], f32r, tag="w1")
                nc.sync.dma_start(out=w1b, in_=w1t[j + JT])
                hts = htp.tile([P, MI], f32r, tag="ht")
                for m in range(MB):
                    rhs = lambda k: xg_tiles[k][:, m * 512:(m + 1) * 512]
                    pb = pp1.tile([P, 512], f32, tag="ps1")
                    for k in range(KT):
                        nc.tensor.matmul(pb, w1b[:, k * P:(k + 1) * P], rhs(k),
                                         start=(k == 0), stop=(k == KT - 1))
                    pa = pp1.tile([P, 512], f32, tag="ps1")
                    for k in range(KT):
                        nc.tensor.matmul(pa, w1a[:, k * P:(k + 1) * P], rhs(k),
                                         start=(k == 0), stop=(k == KT - 1))
                    sil = silp.tile([P, 512], f32, tag="sil")
                    nc.scalar.activation(sil, pb, SIGMOID)
                    nc.vector.tensor_mul(sil, sil, pb)
                    nc.vector.tensor_mul(hts[:, m * 512:(m + 1) * 512], sil, pa)
                nc.sync.dma_start(out=hT[j * P:(j + 1) * P, :], in_=hts)

        with tc.tile_pool(name="w2", bufs=KT) as w2p, \
             tc.tile_pool(name="htm", bufs=2 * KT) as htmp, \
             tc.tile_pool(name="sgp", bufs=1) as sgp, \
             tc.tile_pool(name="yp", bufs=2) as yp, \
             tc.tile_pool(name="ps2", bufs=8, space="PSUM") as pp2:
            sgt = sgp.tile([P, MT], f32)
            nc.sync.dma_start(out=sgt, in_=sg[:, :])
            w2_tiles = []
            for k in range(JT):
                t = w2p.tile([P, D], f32r, tag="w2")
                nc.sync.dma_start(out=t, in_=w2[k * P:(k + 1) * P, :])
                w2_tiles.append(t)
            for mi in range(MT):
                hm = []
                for k in range(JT):
                    t = htmp.tile([P, P], f32r, tag="htm")
                    nc.sync.dma_start(out=t, in_=hT[k * P:(k + 1) * P, mi * P:(mi + 1) * P])
                    hm.append(t)
                yt = yp.tile([P, D], f32, tag="yp")
                for d in range(DB):
                    pd = pp2.tile([P, 512], f32, tag="ps2")
                    for k in range(JT):
                        nc.tensor.matmul(pd, hm[k], w2_tiles[k][:, d * 512:(d + 1) * 512],
                                         start=(k == 0), stop=(k == JT - 1))
                    nc.vector.tensor_scalar_mul(yt[:, d * 512:(d + 1) * 512], pd,
                                                sgt[:, mi:mi + 1])
                nc.sync.dma_start(out=y[mi * P:(mi + 1) * P, :], in_=yt)

    nc.compile()
    return nc


def _build_runner():
    """Compile once; return run(in_maps) -> list of {"y": np.ndarray}."""
    import jax
    import jax.numpy as jnp
    from jax.sharding import Mesh, PartitionSpec, NamedSharding
    from jax.experimental.shard_map import shard_map
    from concourse import bass2jax, mybir

    nc = _build_nc()
    bass2jax.install_neuronx_cc_hook()

    partition_name = nc.partition_id_tensor.name if nc.partition_id_tensor else None
    in_names, out_names, out_avals, zero_outs = [], [], [], []
    for alloc in nc.m.functions[0].allocations:
        if not isinstance(alloc, mybir.MemoryLocationSet):
            continue
        if not alloc.memorylocations:
            continue
        name = alloc.memorylocations[0].name
        if alloc.kind == "ExternalInput":
            if name != partition_name:
                in_names.append(name)
        elif alloc.kind == "ExternalOutput":
            shape = tuple(alloc.tensor_shape)
            dtype = mybir.dt.np(alloc.dtype)
            out_names.append(name)
            out_avals.append(jax.core.ShapedArray(shape, dtype))
            zero_outs.append(np.zeros(shape, dtype))
    n_params = len(in_names)
    n_outs = len(out_names)
    all_in = in_names + out_names
    if partition_name is not None:
        all_in = all_in + [partition_name]

    def _body(*args):
        operands = list(args)
        if partition_name is not None:
            operands.append(bass2jax.partition_id_tensor())
        outs = bass2jax._bass_exec_p.bind(
            *operands,
            out_avals=tuple(out_avals),
            in_names=tuple(all_in),
            out_names=tuple(out_names),
            lowering_input_output_aliases=(),
            sim_require_finite=True,
            sim_require_nnan=True,
            nc=nc,
        )
        return tuple(outs)

    devices = jax.devices()[:N_EXP]
    mesh = Mesh(np.asarray(devices), ("core",))
    spec = PartitionSpec("core")
    sharded = jax.jit(
        shard_map(_body, mesh=mesh, in_specs=(spec,) * (n_params + n_outs),
                  out_specs=(spec,) * n_outs, check_rep=False),
        donate_argnums=tuple(range(n_params, n_params + n_outs)),
        keep_unused=True,
    )
    sharding = NamedSharding(mesh, spec)

    def put_zeros():
        import jax as _jax
        return [_jax.device_put(np.concatenate([z] * N_EXP, axis=0), sharding)
                for z in zero_outs]

    def put_inputs(in_maps):
        import jax as _jax
        cat = [np.concatenate([np.asarray(m[name]) for m in in_maps], axis=0)
               for name in in_names]
        return [_jax.device_put(c, sharding) for c in cat]

    def run_dev(dev_in, dev_zero):
        outs = sharded(*dev_in, *dev_zero)
        return outs

    def split_outs(outs):
        res = [dict() for _ in range(N_EXP)]
        for i, name in enumerate(out_names):
            arr = np.asarray(outs[i])
            per = arr.shape[0] // N_EXP
            for c in range(N_EXP):
                res[c][name] = arr[c * per:(c + 1) * per]
        return res

    return dict(put_zeros=put_zeros, put_inputs=put_inputs, run_dev=run_dev,
                split_outs=split_outs, nc=nc)


def _get_runner():
    if "runner" not in _CACHE:
        _CACHE["runner"] = _build_runner()
    return _CACHE["runner"]


def _prep_in_maps(x, s, Wl1, Wl2, idx):
    idx_r = idx.reshape(N_EXP, MI)
    in_maps = []
    for n in range(N_EXP):
        tok = idx_r[n]
        xgT = np.ascontiguousarray(x[tok].T)
        w1t = np.ascontiguousarray(
            Wl1[n].reshape(KT, P, 2 * JT, P).transpose(2, 1, 0, 3)
        ).reshape(2 * JT, P, KT * P)
        sgv = np.ascontiguousarray(s[n, tok].reshape(MT, P).T)
        in_maps.append({"xgT": xgT, "w1t": w1t, "w2": np.ascontiguousarray(Wl2[n]),
                       "sg": sgv})
    return in_maps


def kernel(x, s, Wl1, Wl2, idx, which, cumsums):
    x = np.asarray(x, np.float32)
    s = np.asarray(s, np.float32)
    Wl1 = np.asarray(Wl1, np.float32)
    Wl2 = np.asarray(Wl2, np.float32)
    idx = np.asarray(idx, np.int32)
    which = np.asarray(which, np.int32)

    r = _get_runner()
    in_maps = _prep_in_maps(x, s, Wl1, Wl2, idx)
    dev_in = r["put_inputs"](in_maps)
    dev_zero = r["put_zeros"]()
    outs = r["run_dev"](dev_in, dev_zero)
    results = r["split_outs"](outs)

    idx_r = idx.reshape(N_EXP, MI)
    which_r = which.reshape(N_EXP, MI)
    out = np.zeros((TOPK, M, D), np.float32)
    for n in range(N_EXP):
        out[which_r[n], idx_r[n]] = results[n]["y"]
    return out


# revision 8
# speedup vs baseline: 70.7651x; 70.7651x over previous
"""MoE expert-FFN (grouped GEMM + SwiGLU) kernel for 8 Trainium2 NeuronCores.

Expert-parallel: expert n lives on core n (N=8 experts, 8 cores).
Host does dispatch (gather tokens per expert) and combine (scatter back);
each core runs lin1 -> SwiGLU -> lin2 -> router-scale for its expert.

Per-core device program (all fp32):
  phase 1: abT[j] = Wl1_cols_j^T @ xg^T accumulated in PSUM per
           (col-block j, token-block m); SwiGLU fuses the a/b halves:
           hT = silu(bT) * aT, spilled to a DRAM scratch [H, Mi].
  phase 2: y[m, :] = (hT[:, m]^T @ Wl2) * sg[m], written [Mi, D].
"""

import sys

sys.path.insert(0, "/opt/trn_rl_repo")

import numpy as np

M, D, H, N_EXP, TOPK = 8192, 2048, 2048, 8, 2
MI = M * TOPK // N_EXP  # 2048 tokens per expert (balanced routing)
P = 128
KT = D // P    # 16 contraction tiles in lin1 (over D)
JT = H // P    # 16 column blocks per a/b half of Wl1
MB = MI // 512  # 4 moving-operand blocks of 512 tokens (phase 1)
DB = D // 512   # 4 output d-blocks of 512 (phase 2)
MT = MI // P   # 16 token tiles (phase 2)

_CACHE = {}


def _build_nc(loop_n: int = 0):
    """loop_n > 1 wraps the body in a device-side For_i loop (benchmarking)."""
    from contextlib import nullcontext
    from concourse import bacc, mybir
    import concourse.tile as tile

    f32 = mybir.dt.float32
    f32r = mybir.dt.float32r
    SIGMOID = mybir.ActivationFunctionType.Sigmoid

    nc = bacc.Bacc(None, target_bir_lowering=False)
    xgT = nc.dram_tensor("xgT", [D, MI], f32r, kind="ExternalInput")
    # w1t[j, kk, k*128+nn] = Wl1[k*128+kk, j*128+nn]; j<16 -> gate (a), j>=16 -> up (b)
    w1t = nc.dram_tensor("w1t", [2 * JT, P, KT * P], f32r, kind="ExternalInput")
    w2 = nc.dram_tensor("w2", [H, D], f32r, kind="ExternalInput")
    sg = nc.dram_tensor("sg", [P, MT], f32, kind="ExternalInput")
    y = nc.dram_tensor("y", [MI, D], f32, kind="ExternalOutput")
    hT = nc.dram_tensor("hT", [H, MI], f32r)  # internal scratch

    with tile.TileContext(nc) as tc:
        with (tc.For_i(0, loop_n, 1) if loop_n > 1 else nullcontext()):
            _emit_body(nc, tc, mybir, xgT, w1t, w2, sg, y, hT)
    nc.compile()
    return nc


def _emit_body(nc, tc, mybir, xgT, w1t, w2, sg, y, hT):
    f32 = mybir.dt.float32
    f32r = mybir.dt.float32r
    SIGMOID = mybir.ActivationFunctionType.Sigmoid
    if True:
        with tc.tile_pool(name="xg", bufs=KT) as xgp, \
             tc.tile_pool(name="w1", bufs=3) as w1p, \
             tc.tile_pool(name="ht", bufs=2) as htp, \
             tc.tile_pool(name="sil", bufs=4) as silp, \
             tc.tile_pool(name="ps1", bufs=8, space="PSUM") as pp1:
            xg_tiles = []
            for k in range(KT):
                t = xgp.tile([P, MI], f32r, tag="xg")
                nc.sync.dma_start(out=t, in_=xgT[k * P:(k + 1) * P, :])
                xg_tiles.append(t)
            for j in range(JT):
                w1a = w1p.tile([P, KT * P], f32r, tag="w1")
                nc.sync.dma_start(out=w1a, in_=w1t[j])
                w1b = w1p.tile([P, KT * P], f32r, tag="w1")
                nc.sync.dma_start(out=w1b, in_=w1t[j + JT])
                hts = htp.tile([P, MI], f32r, tag="ht")
                for m in range(MB):
                    rhs = lambda k: xg_tiles[k][:, m * 512:(m + 1) * 512]
                    pb = pp1.tile([P, 512], f32, tag="ps1")
                    for k in range(KT):
                        nc.tensor.matmul(pb, w1b[:, k * P:(k + 1) * P], rhs(k),
                                         start=(k == 0), stop=(k == KT - 1))
                    pa = pp1.tile([P, 512], f32, tag="ps1")
                    for k in range(KT):
                        nc.tensor.matmul(pa, w1a[:, k * P:(k + 1) * P], rhs(k),
                                         start=(k == 0), stop=(k == KT - 1))
                    sil = silp.tile([P, 512], f32, tag="sil")
                    nc.scalar.activation(sil, pb, SIGMOID)
                    nc.vector.tensor_mul(sil, sil, pb)
                    nc.vector.tensor_mul(hts[:, m * 512:(m + 1) * 512], sil, pa)
                nc.sync.dma_start(out=hT[j * P:(j + 1) * P, :], in_=hts)

        with tc.tile_pool(name="w2", bufs=KT) as w2p, \
             tc.tile_pool(name="htm", bufs=2 * KT) as htmp, \
             tc.tile_pool(name="sgp", bufs=1) as sgp, \
             tc.tile_pool(name="yp", bufs=2) as yp, \
             tc.tile_pool(name="ps2", bufs=8, space="PSUM") as pp2:
            sgt = sgp.tile([P, MT], f32)
            nc.sync.dma_start(out=sgt, in_=sg[:, :])
            w2_tiles = []
            for k in range(JT):
                t = w2p.tile([P, D], f32r, tag="w2")
                nc.sync.dma_start(out=t, in_=w2[k * P:(k + 1) * P, :])
                w2_tiles.append(t)
            for mi in range(MT):
                hm = []
                for k in range(JT):
                    t = htmp.tile([P, P], f32r, tag="htm")
                    nc.sync.dma_start(out=t, in_=hT[k * P:(k + 1) * P, mi * P:(mi + 1) * P])
                    hm.append(t)
                yt = yp.tile([P, D], f32, tag="yp")
                for d in range(DB):
                    pd = pp2.tile([P, 512], f32, tag="ps2")
                    for k in range(JT):
                        nc.tensor.matmul(pd, hm[k], w2_tiles[k][:, d * 512:(d + 1) * 512],
                                         start=(k == 0), stop=(k == JT - 1))
                    nc.vector.tensor_scalar_mul(yt[:, d * 512:(d + 1) * 512], pd,
                                                sgt[:, mi:mi + 1])
                nc.sync.dma_start(out=y[mi * P:(mi + 1) * P, :], in_=yt)


def _build_runner():
    """Compile once; return run(in_maps) -> list of {"y": np.ndarray}."""
    import jax
    import jax.numpy as jnp
    from jax.sharding import Mesh, PartitionSpec, NamedSharding
    from jax.experimental.shard_map import shard_map
    from concourse import bass2jax, mybir

    nc = _build_nc()
    bass2jax.install_neuronx_cc_hook()

    partition_name = nc.partition_id_tensor.name if nc.partition_id_tensor else None
    in_names, out_names, out_avals, zero_outs = [], [], [], []
    for alloc in nc.m.functions[0].allocations:
        if not isinstance(alloc, mybir.MemoryLocationSet):
            continue
        if not alloc.memorylocations:
            continue
        name = alloc.memorylocations[0].name
        if alloc.kind == "ExternalInput":
            if name != partition_name:
                in_names.append(name)
        elif alloc.kind == "ExternalOutput":
            shape = tuple(alloc.tensor_shape)
            dtype = mybir.dt.np(alloc.dtype)
            out_names.append(name)
            out_avals.append(jax.core.ShapedArray(shape, dtype))
            zero_outs.append(np.zeros(shape, dtype))
    n_params = len(in_names)
    n_outs = len(out_names)
    all_in = in_names + out_names
    if partition_name is not None:
        all_in = all_in + [partition_name]

    def _body(*args):
        operands = list(args)
        if partition_name is not None:
            operands.append(bass2jax.partition_id_tensor())
        outs = bass2jax._bass_exec_p.bind(
            *operands,
            out_avals=tuple(out_avals),
            in_names=tuple(all_in),
            out_names=tuple(out_names),
            lowering_input_output_aliases=(),
            sim_require_finite=True,
            sim_require_nnan=True,
            nc=nc,
        )
        return tuple(outs)

    devices = jax.devices()[:N_EXP]
    mesh = Mesh(np.asarray(devices), ("core",))
    spec = PartitionSpec("core")
    sharded = jax.jit(
        shard_map(_body, mesh=mesh, in_specs=(spec,) * (n_params + n_outs),
                  out_specs=(spec,) * n_outs, check_rep=False),
        donate_argnums=tuple(range(n_params, n_params + n_outs)),
        keep_unused=True,
    )
    sharding = NamedSharding(mesh, spec)

    def put_zeros():
        import jax as _jax
        return [_jax.device_put(np.concatenate([z] * N_EXP, axis=0), sharding)
                for z in zero_outs]

    def put_inputs(in_maps):
        import jax as _jax
        cat = [np.concatenate([np.asarray(m[name]) for m in in_maps], axis=0)
               for name in in_names]
        return [_jax.device_put(c, sharding) for c in cat]

    def run_dev(dev_in, dev_zero):
        outs = sharded(*dev_in, *dev_zero)
        return outs

    def split_outs(outs):
        res = [dict() for _ in range(N_EXP)]
        for i, name in enumerate(out_names):
            arr = np.asarray(outs[i])
            per = arr.shape[0] // N_EXP
            for c in range(N_EXP):
                res[c][name] = arr[c * per:(c + 1) * per]
        return res

    return dict(put_zeros=put_zeros, put_inputs=put_inputs, run_dev=run_dev,
                split_outs=split_outs, nc=nc)


def _get_runner():
    if "runner" not in _CACHE:
        _CACHE["runner"] = _build_runner()
    return _CACHE["runner"]


def _prep_in_maps(x, s, Wl1, Wl2, idx):
    idx_r = idx.reshape(N_EXP, MI)
    in_maps = []
    for n in range(N_EXP):
        tok = idx_r[n]
        xgT = np.ascontiguousarray(x[tok].T)
        w1t = np.ascontiguousarray(
            Wl1[n].reshape(KT, P, 2 * JT, P).transpose(2, 1, 0, 3)
        ).reshape(2 * JT, P, KT * P)
        sgv = np.ascontiguousarray(s[n, tok].reshape(MT, P).T)
        in_maps.append({"xgT": xgT, "w1t": w1t, "w2": np.ascontiguousarray(Wl2[n]),
                       "sg": sgv})
    return in_maps


def kernel(x, s, Wl1, Wl2, idx, which, cumsums):
    x = np.asarray(x, np.float32)
    s = np.asarray(s, np.float32)
    Wl1 = np.asarray(Wl1, np.float32)
    Wl2 = np.asarray(Wl2, np.float32)
    idx = np.asarray(idx, np.int32)
    which = np.asarray(which, np.int32)

    r = _get_runner()
    in_maps = _prep_in_maps(x, s, Wl1, Wl2, idx)
    dev_in = r["put_inputs"](in_maps)
    dev_zero = r["put_zeros"]()
    outs = r["run_dev"](dev_in, dev_zero)
    results = r["split_outs"](outs)

    idx_r = idx.reshape(N_EXP, MI)
    which_r = which.reshape(N_EXP, MI)
    out = np.zeros((TOPK, M, D), np.float32)
    for n in range(N_EXP):
        out[which_r[n], idx_r[n]] = results[n]["y"]
    return out
